# revision 1
# baseline (speedup 1.0000x reference)
"""GCN encoder (3x GCNConv + BatchNorm + ReLU) on 8 Trainium2 NeuronCores.

Strategy
--------
GCNConv commutes with the weight matmul:  A_norm (X W) = (A_norm X) W.
With T[i] = dinv[i] * x[i] (the "table"), per layer:
    agg_d = dinv_d * (sum_{e:dst=d} T[src_e] + T[d])     (self-loop folded in)
    z     = agg @ W_l                                     (bias cancels in BN)
    out   = BN(z) (*ReLU for layers 0,1)
    T'    = dinv * out                                    (next layer's table)

Distribution: nodes are degree-sorted and dealt round-robin to 8 cores.  Each
core owns NSH dst nodes, gathers its incident edges' table rows from a full
replicated table in its HBM via dma_gather (bf16 hi/lo split rows, 256B), and
aggregates with one-hot matmuls on the TensorEngine (PSUM accumulation).
Tables are rebuilt per layer: each core computes its shard, AllGather
replicates it.  BatchNorm statistics use a Gram-matrix trick on the PE plus a
tiny AllReduce.

The edge index is preprocessed host-side into a static gather/block schedule
(identical program on all cores; only the data differs).
"""
import sys
sys.path.insert(0, '/opt/trn_rl_repo')

import math
import numpy as np
import ml_dtypes
from contextlib import ExitStack
from dataclasses import dataclass, field

from concourse import bass, bacc, mybir, tile
from concourse.bass_utils import run_bass_kernel_spmd

P = 128           # partitions
D = 64            # feature dim
RW = 2 * D        # table row width in bf16 elems (hi|lo) = 256B
C = 8             # cores
L = 3             # layers
EPS = 1e-5
MAXW = 32512      # max rows addressable by int16 gather index


@dataclass
class Cfg:
    N: int                   # real nodes
    T: int                   # tiles per core
    G: int                   # tiles per supertile
    MB: int = 8              # M-build is_equal batch (blocks)

    @property
    def NSH(self):  return self.T * P          # nodes per core (padded)
    @property
    def NTOT(self): return self.NSH * C        # table rows
    @property
    def NW(self):   return max(1, math.ceil(self.NTOT / MAXW))  # windows
    @property
    def WROWS(self): return self.NTOT // self.NW
    @property
    def NST(self):  return math.ceil(self.T / self.G)


FULL = Cfg(N=100000, T=98, G=2)
NIDX_BLKS = 8     # blocks (128 idxs each) per dma_gather call; >8 (1024 idxs)
                  # passes CoreSim but hung the device when tried on HW


@dataclass
class Schedule:
    B: np.ndarray            # [T, NW] block counts (cross-core max)
    tb_first: np.ndarray     # [T] first (w) with B>0
    tb_last: np.ndarray      # [T] last (w) with B>0
    totb: int                # total blocks
    totidx: int              # total gather idxs (= totb*128)
    grp_ng: list             # per (st, w): n blocks
    grp_ioff: list           # per (st, w): idx col offset (int16 cols)
    blk_col: np.ndarray      # [T, NW] -> first dstloc column of (t, w)
    eg_boff: np.ndarray      # [T, NW] -> block offset inside its (st,w) gather


def build_schedule(cfg: Cfg, counts_max: np.ndarray) -> Schedule:
    """counts_max: [T, NW] max-over-cores edge counts per (tile, window)."""
    T, NW, G = cfg.T, cfg.NW, cfg.G
    B = np.ceil(counts_max / P).astype(np.int64)
    # every tile must have >=1 block so PSUM has a defined accumulation group
    if (B.sum(axis=1) == 0).any():
        B[B.sum(axis=1) == 0, 0] = 1
    tb_first = np.array([np.flatnonzero(B[t])[0] for t in range(T)])
    tb_last = np.array([np.flatnonzero(B[t])[-1] for t in range(T)])

    # schedule order: for st, for w, for t in st, blocks
    blk_col = np.zeros((T, NW), np.int64)
    eg_boff = np.zeros((T, NW), np.int64)
    grp_ng, grp_ioff = [], []
    col = 0
    for st in range(cfg.NST):
        ts = range(st * G, min((st + 1) * G, T))
        for w in range(NW):
            ng = 0
            for t in ts:
                blk_col[t, w] = col
                eg_boff[t, w] = ng
                ng += B[t, w]
                col += B[t, w]
            grp_ng.append(int(ng))
            grp_ioff.append(None)  # filled below
    totb = int(B.sum())
    # idx col offsets: groups are laid out consecutively; each block = 128 idx
    # = 8 int16 cols
    ioff = 0
    for i, ng in enumerate(grp_ng):
        grp_ioff[i] = ioff
        ioff += ng * 8
    return Schedule(B=B, tb_first=tb_first, tb_last=tb_last, totb=totb,
                    totidx=totb * P, grp_ng=grp_ng, grp_ioff=grp_ioff,
                    blk_col=blk_col, eg_boff=eg_boff)


def preprocess(cfg: Cfg, x: np.ndarray, edge_index: np.ndarray):
    """Host-side graph partitioning -> per-core inputs + static schedule."""
    N, T, NW, G = cfg.N, cfg.T, cfg.NW, cfg.G
    NSH, WROWS = cfg.NSH, cfg.WROWS
    src, dst = edge_index[0].astype(np.int64), edge_index[1].astype(np.int64)

    deg = np.bincount(dst, minlength=N).astype(np.float64) + 1.0
    dinv = (1.0 / np.sqrt(deg)).astype(np.float32)

    order = np.argsort(deg, kind="stable")
    rank = np.empty(N, np.int64)
    rank[order] = np.arange(N)
    core_of = (rank % C).astype(np.int64)
    kidx = (rank // C).astype(np.int64)         # rank within core (dense)

    local_of = kidx
    pos = core_of * NSH + local_of              # table row of node

    NREAL = N // C                              # real nodes per core
    assert N % C == 0, "N must divide by 8"
    assert NREAL < NSH, "need pad (zero) rows in every shard"
    assert (C % NW == 0) and WROWS == (C // NW) * NSH, "windows must tile shards"
    # window w starts at core (C//NW)*w's shard, so local offset PAD_SL is a
    # pad (zero) row within every window
    PAD_SL = NREAL

    # edge stream = graph edges + self edges
    ec = np.concatenate([core_of[dst], core_of])
    et = np.concatenate([local_of[dst] // P, local_of // P])
    ep = np.concatenate([local_of[dst] % P, local_of % P]).astype(np.int64)
    ew = np.concatenate([pos[src] // WROWS, pos // WROWS])
    esl = np.concatenate([pos[src] % WROWS, pos % WROWS]).astype(np.int64)

    key = ((ec * T + et) * NW + ew)
    o2 = np.argsort(key, kind="stable")
    key_s, ep_s, esl_s = key[o2], ep[o2], esl[o2]
    counts = np.bincount(key_s, minlength=C * T * NW).reshape(C, T, NW)
    sched = build_schedule(cfg, counts.max(axis=0))
    B = sched.B

    # rank within group
    grp_start = np.concatenate([[0], np.cumsum(np.bincount(key_s, minlength=C * T * NW))])
    rwithin = np.arange(len(key_s)) - grp_start[key_s]

    # linear position of each (t, w) group's padded stream, shared by cores
    stream_off = np.zeros((T, NW), np.int64)
    off = 0
    for st in range(cfg.NST):
        ts = range(st * G, min((st + 1) * G, T))
        for w in range(NW):
            for t in ts:
                stream_off[t, w] = off
                off += B[t, w] * P
    assert off == sched.totidx

    ecore = key_s // (T * NW)
    etw = key_s % (T * NW)
    lin = stream_off[etw // NW, etw % NW] + rwithin

    xs, idxs, dstlocs, dinvs = [], [], [], []
    for c in range(C):
        m = ecore == c
        sl_pad = np.full(sched.totidx, PAD_SL, np.int64)
        p_pad = np.zeros(sched.totidx, np.int64)
        sl_pad[lin[m]] = esl_s[m]
        p_pad[lin[m]] = ep_s[m]

        # wrap idx: j -> partition j%16 (x8 replicas), col j//16
        colsn = sched.totidx // 16
        w16 = sl_pad.reshape(colsn, 16).T.astype(np.int16)
        idxs.append(np.tile(w16, (8, 1)))
        # dstloc [128, totb]
        dstlocs.append(np.ascontiguousarray(
            p_pad.reshape(sched.totb, P).T).astype(np.float32))

        nodes_c = order[c::C]                   # in local order
        xc = np.zeros((NSH, D), np.float32)
        xc[:len(nodes_c)] = x[nodes_c]
        xs.append(xc)
        dc = np.zeros((NSH, 1), np.float32)
        dc[:len(nodes_c), 0] = dinv[nodes_c]
        dinvs.append(np.ascontiguousarray(dc.reshape(T, P).T))  # [128, T]

    return sched, xs, idxs, dstlocs, dinvs, core_of, local_of


def dram_alias(nc, base_handle, name, shape, dtype, byte_off):
    h = nc.dram_tensor(name, shape, dtype)
    base_addr = nc.lookup_mloc(base_handle).addr
    nc.lookup_mls(h).memorylocations[0].addr = base_addr + byte_off
    return h


def build_program(cfg: Cfg, sched: Schedule, alias_windows=True):
    import os
    DBG_LAYERS = int(os.environ.get("GCN_LAYERS", str(L)))
    DBG_SKIP_AR = os.environ.get("GCN_SKIP_AR", "0") == "1"
    DBG_SKIP_GATHER = os.environ.get("GCN_SKIP_GATHER", "0") == "1"
    DBG_SKIP_MM = os.environ.get("GCN_SKIP_MM", "0") == "1"
    DBG_SKIP_TILEOPS = os.environ.get("GCN_SKIP_TILEOPS", "0") == "1"
    DBG_SKIP_PASSE = os.environ.get("GCN_SKIP_PASSE", "0") == "1"

    T, NW, G, MB = cfg.T, cfg.NW, cfg.G, cfg.MB
    NSH, NTOT, WROWS, NST = cfg.NSH, cfg.NTOT, cfg.WROWS, cfg.NST
    bf16, f32, i16 = mybir.dt.bfloat16, mybir.dt.float32, mybir.dt.int16
    AF = mybir.ActivationFunctionType
    OP = mybir.AluOpType
    icols = sched.totidx // 16

    nc = bacc.Bacc("TRN2", target_bir_lowering=False, debug=False, num_devices=C)

    # I/O
    x_in = nc.dram_tensor("x", [NSH, D], f32, kind="ExternalInput")
    idx_in = nc.dram_tensor("idxs", [P, icols], i16, kind="ExternalInput")
    dl_in = nc.dram_tensor("dstloc", [P, sched.totb], f32, kind="ExternalInput")
    dv_in = nc.dram_tensor("dinv", [P, T], f32, kind="ExternalInput")
    w_in = nc.dram_tensor("Wcat", [D, L * D], f32, kind="ExternalInput")
    gb_in = nc.dram_tensor("gbcat", [D, 2 * L], f32, kind="ExternalInput")
    io_in = nc.dram_tensor("iota", [1, P], bf16, kind="ExternalInput")
    id_in = nc.dram_tensor("ident", [P, P], f32, kind="ExternalInput")
    out_d = nc.dram_tensor("out", [NSH, D], f32, kind="ExternalOutput")

    # internal DRAM
    tloc = nc.dram_tensor("tloc", [NSH, RW], bf16)
    tfull = nc.dram_tensor("tfull", [NTOT, RW], bf16)
    if alias_windows:
        wins = [dram_alias(nc, tfull, f"win{w}", [WROWS, RW], bf16,
                           w * WROWS * RW * 2)
                for w in range(NW)]
    else:  # sim only: offset APs (crash real HW, fine in CoreSim)
        wins = [tfull[w * WROWS:(w + 1) * WROWS, :] for w in range(NW)]
    ar_in = nc.dram_tensor("ar_in", [D + 1, D + 1], f32)
    ar_out = nc.dram_tensor("ar_out", [D + 1, D + 1], f32)
    st_d = nc.dram_tensor("st_d", [2, D], f32)

    # persistent SBUF
    zsh = nc.alloc_sbuf_tensor("zsh", [P, T * (D + 1)], f32).ap()
    idx_sb = nc.alloc_sbuf_tensor("idx_sb", [P, icols], i16).ap()
    dl_sb = nc.alloc_sbuf_tensor("dl_sb", [P, sched.totb], f32).ap()
    dv_sb = nc.alloc_sbuf_tensor("dv_sb", [P, T], f32).ap()
    w_sb = nc.alloc_sbuf_tensor("w_sb", [D, L * D], f32).ap()
    gb_sb = nc.alloc_sbuf_tensor("gb_sb", [D, 2 * L], f32).ap()
    io_bc = nc.alloc_sbuf_tensor("io_bc", [P, P], bf16).ap()
    id_sb = nc.alloc_sbuf_tensor("id_sb", [P, P], f32).ap()
    ones_sb = nc.alloc_sbuf_tensor("ones_sb", [P, 1], f32).ap()
    st_row = nc.alloc_sbuf_tensor("st_row", [2, D], f32).ap()
    st_bc = nc.alloc_sbuf_tensor("st_bc", [P, 2 * D], f32).ap()
    stat_sb = nc.alloc_sbuf_tensor("stat_sb", [D + 1, D + 1], f32).ap()

    with tile.TileContext(nc) as tc:
        with ExitStack() as ctx:
            sb = ctx.enter_context(tc.tile_pool(name="sb", bufs=3))
            egp = ctx.enter_context(tc.tile_pool(name="eg", bufs=int(__import__("os").environ.get("EGB", "6"))))
            mp = ctx.enter_context(tc.tile_pool(name="mp", bufs=4))
            pp = ctx.enter_context(tc.tile_pool(name="pp", bufs=3, space="PSUM"))
            pt = ctx.enter_context(tc.tile_pool(name="pt", bufs=2, space="PSUM"))
            pz = ctx.enter_context(tc.tile_pool(name="pz", bufs=2, space="PSUM"))
            ps = ctx.enter_context(tc.tile_pool(name="ps", bufs=1, space="PSUM"))

            # ---- prologue: consts + layer-0 table ----
            nc.sync.dma_start(out=idx_sb[:], in_=idx_in[:, :])
            nc.sync.dma_start(out=dl_sb[:], in_=dl_in[:, :])
            nc.sync.dma_start(out=dv_sb[:], in_=dv_in[:, :])
            nc.sync.dma_start(out=w_sb[:], in_=w_in[:, :])
            nc.sync.dma_start(out=gb_sb[:], in_=gb_in[:, :])
            nc.gpsimd.dma_start(out=io_bc[:], in_=bass.AP(
                io_in.ap().tensor, 0, [[0, P], [1, P]]))
            nc.sync.dma_start(out=id_sb[:], in_=id_in[:, :])
            nc.vector.memset(ones_sb[:], 1.0)
            nc.vector.memset(
                zsh.rearrange("p (t e) -> p t e", t=T)[:, :, D:D + 1], 1.0)

            for t in range(T):
                xt = sb.tile([P, D], f32, tag="xt")
                nc.sync.dma_start(out=xt[:], in_=x_in[t * P:(t + 1) * P, :])
                r3 = sb.tile([P, D], f32, tag="r3")
                nc.vector.tensor_scalar(
                    out=r3[:], in0=xt[:], scalar1=dv_sb[:, t:t + 1],
                    scalar2=None, op0=OP.mult)
                tl = sb.tile([P, RW], bf16, tag="tl")
                nc.scalar.activation(tl[:, :D], r3[:], AF.Copy)
                nc.vector.tensor_tensor(
                    out=tl[:, D:], in0=r3[:], in1=tl[:, :D], op=OP.subtract)
                nc.sync.dma_start(out=tloc[t * P:(t + 1) * P, :], in_=tl[:])

            # ---- layers ----
            for l in range(DBG_LAYERS):
                nc.gpsimd.collective_compute(
                    "AllGather", OP.bypass,
                    ins=[tloc.ap().opt()],
                    outs=[tfull.ap().opt()],
                    replica_groups=[list(range(C))],
                )
                tc.strict_bb_all_engine_barrier()   # alias deps: tfull -> wins
                gram_ps = ps.tile([D + 1, D + 1], f32, tag="stats")
                gi = 0  # group index
                if DBG_SKIP_GATHER:
                    nc.sync.dma_start(out=out_d[0:P, :], in_=st_bc[:, 0:D])
                    continue
                for st in range(NST):
                    ts = list(range(st * G, min((st + 1) * G, T)))
                    egs = {}
                    for w in range(NW):
                        ng = sched.grp_ng[gi + w]
                        if ng == 0:
                            continue
                        eg = egp.tile([P, ng * RW], bf16, tag="eg")
                        eg3 = eg[:].rearrange("p (b e) -> p b e", b=ng)
                        ioff = sched.grp_ioff[gi + w]
                        # chunk the group's blocks into dma_gather calls
                        for g0 in range(0, ng, NIDX_BLKS):
                            nb2 = min(NIDX_BLKS, ng - g0)
                            nc.gpsimd.dma_gather(
                                out_ap=eg3[:, g0:g0 + nb2, :],
                                in_ap=wins[w][:, :] if alias_windows else wins[w],
                                idxs_ap=idx_sb[:, ioff + g0 * 8:
                                               ioff + (g0 + nb2) * 8],
                                num_idxs=nb2 * P,
                                num_idxs_reg=nb2 * P,
                                elem_size=RW,
                            )
                        egs[w] = eg
                    if DBG_SKIP_MM:
                        continue

                    # M blocks for this supertile, batched is_equal
                    c0 = sched.blk_col[ts[0], 0]
                    c1 = c0 + sum(sched.grp_ng[gi + w] for w in range(NW))
                    mts = {}
                    for mb0 in range(c0, c1, MB):
                        nb = min(MB, c1 - mb0)
                        mt = mp.tile([P, MB * P], bf16, tag="m")
                        for j in range(nb):
                            nc.vector.tensor_scalar(
                                out=mt[:, j * P:(j + 1) * P],
                                in0=io_bc[:],
                                scalar1=dl_sb[:, mb0 + j:mb0 + j + 1],
                                scalar2=None, op0=OP.is_equal)
                        for j in range(nb):
                            mts[mb0 + j] = (mt, j)

                    for t in ts:
                        agg = pp.tile([P, RW], f32, tag="agg")
                        if DBG_SKIP_TILEOPS and t > 0:
                            continue
                        for w in range(NW):
                            for b in range(sched.B[t, w]):
                                col = sched.blk_col[t, w] + b
                                mt, j = mts[col]
                                bidx = sched.eg_boff[t, w] + b
                                nc.tensor.matmul(
                                    out=agg[:],
                                    lhsT=mt[:, j * P:(j + 1) * P],
                                    rhs=egs[w][:, bidx * RW:(bidx + 1) * RW],
                                    start=(w == sched.tb_first[t] and b == 0),
                                    stop=(w == sched.tb_last[t]
                                          and b == sched.B[t, w] - 1),
                                )
                        # acc2 = (hi + lo) * dinv  (dinv fused into the
                        # PSUM->SBUF copy; TT cannot read 2 PSUM operands)
                        agf = sb.tile([P, RW], f32, tag="agf")
                        nc.scalar.activation(agf[:], agg[:], AF.Copy,
                                             scale=dv_sb[:, t:t + 1])
                        acc2 = sb.tile([P, D], f32, tag="acc2")
                        nc.vector.tensor_tensor(
                            out=acc2[:], in0=agf[:, :D], in1=agf[:, D:], op=OP.add)
                        # z = acc2 @ W_l  (via PE transpose)
                        trp = pt.tile([D, P], f32, tag="tr")
                        nc.tensor.transpose(trp[:], acc2[:], id_sb[:])
                        accT = sb.tile([D, P], f32, tag="accT")
                        nc.scalar.activation(accT[:], trp[:], AF.Copy)
                        zp = pz.tile([P, D], f32, tag="z")
                        nc.tensor.matmul(
                            out=zp[:], lhsT=accT[:],
                            rhs=w_sb[:, l * D:(l + 1) * D],
                            start=True, stop=True)
                        nc.scalar.activation(
                            zsh[:, t * (D + 1):t * (D + 1) + D], zp[:], AF.Copy)
                        # stats: [z|1].T @ [z|1] -> gram + sums in one chain
                        zaug = zsh[:, t * (D + 1):(t + 1) * (D + 1)]
                        nc.tensor.matmul(
                            out=gram_ps[:],
                            lhsT=zaug, rhs=zaug,
                            start=(t == 0), stop=(t == T - 1))
                    gi += NW

                # ---- BN stats: AllReduce + affine coefficients ----
                nc.vector.tensor_copy(out=stat_sb[:], in_=gram_ps[:])
                if not DBG_SKIP_AR:
                    nc.sync.dma_start(out=ar_in[:, :], in_=stat_sb[:])
                    nc.gpsimd.collective_compute(
                        "AllReduce", OP.add,
                        ins=[ar_in.ap().opt()],
                        outs=[ar_out.ap().opt()],
                        replica_groups=[list(range(C))],
                    )
                    nc.sync.dma_start(out=stat_sb[:], in_=ar_out[:, :])

                mu = sb.tile([D, 1], f32, tag="v1")
                nc.vector.tensor_scalar(out=mu[:], in0=stat_sb[:D, D:D + 1],
                                        scalar1=1.0 / cfg.N, scalar2=None,
                                        op0=OP.mult)
                dg = sb.tile([D, D], f32, tag="dg")
                nc.vector.tensor_tensor(out=dg[:], in0=stat_sb[:D, :D],
                                        in1=id_sb[:D, :D], op=OP.mult)
                msq = sb.tile([D, 1], f32, tag="v2")
                nc.vector.tensor_reduce(out=msq[:], in_=dg[:],
                                        axis=mybir.AxisListType.X, op=OP.add)
                var = sb.tile([D, 1], f32, tag="v3")
                # var = msq/N - mu^2 ; then rstd = 1/sqrt(var+eps)
                musq = sb.tile([D, 1], f32, tag="v4")
                nc.vector.tensor_tensor(out=musq[:], in0=mu[:], in1=mu[:],
                                        op=OP.mult)
                nc.vector.tensor_scalar(out=var[:], in0=msq[:],
                                        scalar1=1.0 / cfg.N, scalar2=None,
                                        op0=OP.mult)
                nc.vector.tensor_tensor(out=var[:], in0=var[:], in1=musq[:],
                                        op=OP.subtract)
                sd = sb.tile([D, 1], f32, tag="v5")
                nc.vector.tensor_scalar(out=sd[:], in0=var[:], scalar1=EPS,
                                        scalar2=None, op0=OP.add)
                nc.scalar.activation(sd[:], sd[:], AF.Sqrt)
                rstd = sb.tile([D, 1], f32, tag="v6")
                nc.vector.reciprocal(rstd[:], sd[:])
                stc = sb.tile([D, 2], f32, tag="stc")
                nc.vector.tensor_tensor(out=stc[:, 0:1], in0=gb_sb[:, l:l + 1],
                                        in1=rstd[:], op=OP.mult)
                nc.vector.tensor_tensor(out=stc[:, 1:2], in0=mu[:],
                                        in1=stc[:, 0:1], op=OP.mult)
                nc.vector.tensor_tensor(out=stc[:, 1:2],
                                        in0=gb_sb[:, L + l:L + l + 1],
                                        in1=stc[:, 1:2], op=OP.subtract)
                stp = pt.tile([2, D], f32, tag="tr")
                nc.tensor.transpose(stp[:], stc[:], id_sb[:D, :D])
                nc.scalar.activation(st_row[:], stp[:], AF.Copy)
                nc.sync.dma_start(out=st_d[:, :], in_=st_row[:])
                nc.gpsimd.dma_start(out=st_bc[:], in_=bass.AP(
                    st_d.ap().tensor, 0, [[0, P], [D, 2], [1, D]]))

                # ---- pass E: affine (+relu, + next table) ----
                srow = st_bc[:, 0:D]
                trow = st_bc[:, D:2 * D]
                for t in range(T if not DBG_SKIP_PASSE else 1):
                    zt = zsh[:, t * (D + 1):t * (D + 1) + D]
                    r1 = sb.tile([P, D], f32, tag="r1")
                    nc.vector.tensor_tensor(out=r1[:], in0=zt, in1=srow,
                                            op=OP.mult)
                    r2 = sb.tile([P, D], f32, tag="r2")
                    nc.vector.tensor_tensor(out=r2[:], in0=r1[:], in1=trow,
                                            op=OP.add)
                    if l < L - 1:
                        r3 = sb.tile([P, D], f32, tag="r3")
                        nc.vector.tensor_scalar(
                            out=r3[:], in0=r2[:], scalar1=0.0,
                            scalar2=dv_sb[:, t:t + 1], op0=OP.max, op1=OP.mult)
                        tl = sb.tile([P, RW], bf16, tag="tl")
                        nc.scalar.activation(tl[:, :D], r3[:], AF.Copy)
                        nc.vector.tensor_tensor(out=tl[:, D:], in0=r3[:],
                                                in1=tl[:, :D], op=OP.subtract)
                        nc.sync.dma_start(out=tloc[t * P:(t + 1) * P, :],
                                          in_=tl[:])
                    else:
                        nc.sync.dma_start(out=out_d[t * P:(t + 1) * P, :],
                                          in_=r2[:])
    nc.finalize()
    return nc


_PROG_CACHE = {}


def _get_prog(cfg, sched_key, sched):
    if sched_key not in _PROG_CACHE:
        _PROG_CACHE[sched_key] = build_program(cfg, sched)
    return _PROG_CACHE[sched_key]


def prepare(cfg: Cfg, x, edge_index, Ws, gammas, betas, alias_windows=True):
    """Host preprocess + program build.  Returns (nc, in_maps, unshard)."""
    sched, xs, idxs, dstlocs, dinvs, core_of, local_of = preprocess(
        cfg, np.asarray(x, np.float32), np.asarray(edge_index))
    Wcat = np.ascontiguousarray(
        np.concatenate([np.asarray(Ws[i], np.float32) for i in range(L)], axis=1))
    gbcat = np.ascontiguousarray(np.stack(
        [np.asarray(gammas[i], np.float32) for i in range(L)]
        + [np.asarray(betas[i], np.float32) for i in range(L)], axis=1))
    iota = np.arange(P, dtype=np.float32).reshape(1, P).astype(ml_dtypes.bfloat16)
    ident = np.eye(P, dtype=np.float32)

    key = (cfg.N, cfg.T, cfg.G, sched.totb, sched.totidx,
           tuple(sched.grp_ng), sched.B.tobytes(), alias_windows)
    if key not in _PROG_CACHE:
        _PROG_CACHE[key] = build_program(cfg, sched, alias_windows=alias_windows)
    nc = _PROG_CACHE[key]

    in_maps = []
    for c in range(C):
        in_maps.append({
            "x": xs[c], "idxs": idxs[c], "dstloc": dstlocs[c],
            "dinv": dinvs[c], "Wcat": Wcat, "gbcat": gbcat,
            "iota": iota, "ident": ident,
        })

    def unshard(outs_by_core):
        outs = np.stack([np.asarray(o) for o in outs_by_core])
        return outs[core_of, local_of]          # [N, 64]

    return nc, in_maps, unshard


def run(cfg: Cfg, x, edge_index, Ws, gammas, betas, trace=False):
    nc, in_maps, unshard = prepare(cfg, x, edge_index, Ws, gammas, betas)
    res = run_bass_kernel_spmd(nc, in_maps, core_ids=list(range(C)),
                               trace=trace)
    out_full = unshard([res.results[c]["out"] for c in range(C)])
    return out_full, res


def kernel(x, edge_index, Ws, bs, gammas, betas):
    out, _ = run(FULL, x, edge_index, Ws, gammas, betas, trace=False)
    return out



# revision 5
# speedup vs baseline: 11.6503x; 11.6503x over previous
"""GCN encoder (3x GCNConv + BatchNorm + ReLU) on 8 Trainium2 NeuronCores.

Strategy
--------
GCNConv commutes with the weight matmul:  A_norm (X W) = (A_norm X) W.
With T[i] = dinv[i] * x[i] (the "table"), per layer:
    agg_d = dinv_d * (sum_{e:dst=d} T[src_e] + T[d])     (self-loop folded in)
    z     = agg @ W_l                                     (bias cancels in BN)
    out   = BN(z) (*ReLU for layers 0,1)
    T'    = dinv * out                                    (next layer's table)

Distribution: nodes are degree-sorted and dealt round-robin to 8 cores.  Each
core owns NSH dst nodes, gathers its incident edges' table rows from a full
replicated table in its HBM via dma_gather (bf16 hi/lo split rows, 256B), and
aggregates with one-hot matmuls on the TensorEngine (PSUM accumulation).
Tables are rebuilt per layer: each core computes its shard, AllGather
replicates it.  BatchNorm statistics use a Gram-matrix trick on the PE plus a
tiny AllReduce.  The self-loop term is added from an SBUF-resident strip
instead of flowing through the gather.  The layer-0 table is prepared
host-side (dinv*x, hi/lo split), so the device prologue is two bulk DMAs.

The edge index is preprocessed host-side into a static gather/block schedule
(identical program on all cores; only the data differs).  Within each
(tile, window) group the gather indices are sorted by source row for HBM
locality.
"""
import sys
sys.path.insert(0, '/opt/trn_rl_repo')

import math
import os
import numpy as np
import ml_dtypes
from contextlib import ExitStack
from dataclasses import dataclass, field

from concourse import bass, bacc, mybir, tile
from concourse.bass_utils import run_bass_kernel_spmd

P = 128           # partitions
D = 64            # feature dim
RW = 2 * D        # table row width in bf16 elems (hi|lo) = 256B
C = 8             # cores
L = 3             # layers
EPS = 1e-5
MAXW = 32512      # max rows addressable by int16 gather index


@dataclass
class Cfg:
    N: int                   # real nodes
    T: int                   # tiles per core
    G: int                   # tiles per supertile
    MB: int = 8              # M-build batch (blocks per DVE instruction)
    BW: int = 7              # tiles per batched table/out DMA write

    @property
    def NSH(self):  return self.T * P          # nodes per core (padded)
    @property
    def NTOT(self): return self.NSH * C        # table rows
    @property
    def NW(self):   return max(1, math.ceil(self.NTOT / MAXW))  # windows
    @property
    def WROWS(self): return self.NTOT // self.NW
    @property
    def NST(self):  return math.ceil(self.T / self.G)


FULL = Cfg(N=100000, T=98, G=2)
NIDX_BLKS = int(os.environ.get("NIDX_BLKS", "8"))
                  # blocks (128 idxs each) per dma_gather call; >8 (1024 idxs)
                  # passes CoreSim but hung the device when tried on HW


@dataclass
class Schedule:
    B: np.ndarray            # [T, NW] block counts (cross-core max)
    tb_first: np.ndarray     # [T] first (w) with B>0
    tb_last: np.ndarray      # [T] last (w) with B>0
    totb: int                # total blocks
    totidx: int              # total gather idxs (= totb*128)
    grp_ng: list             # per (st, w): n blocks
    grp_ioff: list           # per (st, w): idx col offset (int16 cols)
    blk_col: np.ndarray      # [T, NW] -> first dstloc column of (t, w)
    eg_boff: np.ndarray      # [T, NW] -> block offset inside its (st,w) gather


def build_schedule(cfg: Cfg, counts_max: np.ndarray) -> Schedule:
    """counts_max: [T, NW] max-over-cores edge counts per (tile, window)."""
    T, NW, G = cfg.T, cfg.NW, cfg.G
    B = np.ceil(counts_max / P).astype(np.int64)
    # every tile must have >=1 block so PSUM has a defined accumulation group
    if (B.sum(axis=1) == 0).any():
        B[B.sum(axis=1) == 0, 0] = 1
    tb_first = np.array([np.flatnonzero(B[t])[0] for t in range(T)])
    tb_last = np.array([np.flatnonzero(B[t])[-1] for t in range(T)])

    # schedule order: for st, for w, for t in st, blocks
    blk_col = np.zeros((T, NW), np.int64)
    eg_boff = np.zeros((T, NW), np.int64)
    grp_ng, grp_ioff = [], []
    col = 0
    for st in range(cfg.NST):
        ts = range(st * G, min((st + 1) * G, T))
        for w in range(NW):
            ng = 0
            for t in ts:
                blk_col[t, w] = col
                eg_boff[t, w] = ng
                ng += B[t, w]
                col += B[t, w]
            grp_ng.append(int(ng))
            grp_ioff.append(None)  # filled below
    totb = int(B.sum())
    # idx col offsets: groups are laid out consecutively; each block = 128 idx
    # = 8 int16 cols
    ioff = 0
    for i, ng in enumerate(grp_ng):
        grp_ioff[i] = ioff
        ioff += ng * 8
    return Schedule(B=B, tb_first=tb_first, tb_last=tb_last, totb=totb,
                    totidx=totb * P, grp_ng=grp_ng, grp_ioff=grp_ioff,
                    blk_col=blk_col, eg_boff=eg_boff)


def _hilo(a):
    """f32 array -> concatenated bf16 hi|lo along last axis."""
    hi = a.astype(ml_dtypes.bfloat16)
    lo = (a - hi.astype(np.float32)).astype(ml_dtypes.bfloat16)
    return np.concatenate([hi, lo], axis=-1)


def preprocess(cfg: Cfg, x: np.ndarray, edge_index: np.ndarray):
    """Host-side graph partitioning -> per-core inputs + static schedule."""
    N, T, NW, G = cfg.N, cfg.T, cfg.NW, cfg.G
    NSH, WROWS = cfg.NSH, cfg.WROWS
    src, dst = edge_index[0].astype(np.int64), edge_index[1].astype(np.int64)

    deg = np.bincount(dst, minlength=N).astype(np.float64) + 1.0
    dinv = (1.0 / np.sqrt(deg)).astype(np.float32)

    order = np.argsort(deg, kind="stable")
    rank = np.empty(N, np.int64)
    rank[order] = np.arange(N)
    core_of = (rank % C).astype(np.int64)
    kidx = (rank // C).astype(np.int64)         # rank within core (dense)

    local_of = kidx
    pos = core_of * NSH + local_of              # table row of node

    NREAL = N // C                              # real nodes per core
    assert N % C == 0, "N must divide by 8"
    assert NREAL < NSH, "need pad (zero) rows in every shard"
    assert (C % NW == 0) and WROWS == (C // NW) * NSH, "windows must tile shards"
    # window w starts at core (C//NW)*w's shard, so local offset PAD_SL is a
    # pad (zero) row within every window
    PAD_SL = NREAL

    # edge stream = graph edges only; self-loops are added on-device from the
    # SBUF-resident self strip
    ec = core_of[dst]
    et = local_of[dst] // P
    ep = (local_of[dst] % P).astype(np.int64)
    ew = pos[src] // WROWS
    esl = (pos[src] % WROWS).astype(np.int64)

    key = ((ec * T + et) * NW + ew)
    o2 = np.lexsort((esl, key))                 # src-sorted within groups
    key_s, ep_s, esl_s = key[o2], ep[o2], esl[o2]
    counts = np.bincount(key_s, minlength=C * T * NW).reshape(C, T, NW)
    sched = build_schedule(cfg, counts.max(axis=0))
    B = sched.B

    # rank within group
    grp_start = np.concatenate([[0], np.cumsum(np.bincount(key_s, minlength=C * T * NW))])
    rwithin = np.arange(len(key_s)) - grp_start[key_s]

    # linear position of each (t, w) group's padded stream, shared by cores
    stream_off = np.zeros((T, NW), np.int64)
    off = 0
    for st in range(cfg.NST):
        ts = range(st * G, min((st + 1) * G, T))
        for w in range(NW):
            for t in ts:
                stream_off[t, w] = off
                off += B[t, w] * P
    assert off == sched.totidx

    ecore = key_s // (T * NW)
    etw = key_s % (T * NW)
    lin = stream_off[etw // NW, etw % NW] + rwithin

    xs, idxs, dstlocs, dinvs, sshs = [], [], [], [], []
    for c in range(C):
        m = ecore == c
        sl_pad = np.full(sched.totidx, PAD_SL, np.int64)
        p_pad = np.zeros(sched.totidx, np.int64)
        sl_pad[lin[m]] = esl_s[m]
        p_pad[lin[m]] = ep_s[m]

        # wrap idx: j -> partition j%16 (x8 replicas), col j//16
        colsn = sched.totidx // 16
        w16 = sl_pad.reshape(colsn, 16).T.astype(np.int16)
        idxs.append(np.tile(w16, (8, 1)))
        # dstloc [128, totb]
        dstlocs.append(np.ascontiguousarray(
            p_pad.reshape(sched.totb, P).T).astype(np.float32))

        nodes_c = order[c::C]                   # in local order
        # layer-0 table rows (dinv*x, hi|lo bf16), host-prepared
        tbl = np.zeros((NSH, D), np.float32)
        tbl[:len(nodes_c)] = x[nodes_c] * dinv[nodes_c, None]
        xs.append(np.ascontiguousarray(_hilo(tbl)))          # [NSH, RW] bf16
        # self strip: ssh[p, t*D+f] = dinv^2 * x  (tile layout)
        s = tbl * 0.0
        s[:len(nodes_c)] = tbl[:len(nodes_c)] * dinv[nodes_c, None]
        sshs.append(np.ascontiguousarray(
            s.reshape(T, P, D).transpose(1, 0, 2).reshape(P, T * D)))
        dc = np.zeros((NSH, 1), np.float32)
        dc[:len(nodes_c), 0] = dinv[nodes_c]
        dinvs.append(np.ascontiguousarray(dc.reshape(T, P).T))  # [128, T]

    return sched, xs, idxs, dstlocs, dinvs, sshs, core_of, local_of


def dram_alias(nc, base_handle, name, shape, dtype, byte_off, addr_space="Local"):
    h = nc.dram_tensor(name, shape, dtype, addr_space=addr_space)
    base_addr = nc.lookup_mloc(base_handle).addr
    nc.lookup_mls(h).memorylocations[0].addr = base_addr + byte_off
    return h


def build_program(cfg: Cfg, sched: Schedule, alias_windows=True, reps=1):
    DBG_LAYERS = int(os.environ.get("GCN_LAYERS", str(L)))
    DBG_SKIP_AR = os.environ.get("GCN_SKIP_AR", "0") == "1"
    DBG_SKIP_GATHER = os.environ.get("GCN_SKIP_GATHER", "0") == "1"
    DBG_SKIP_MM = os.environ.get("GCN_SKIP_MM", "0") == "1"
    BATCH_M = os.environ.get("GCN_BATCH_M", "1") == "1"

    T, NW, G, MB, BW = cfg.T, cfg.NW, cfg.G, cfg.MB, cfg.BW
    NSH, NTOT, WROWS, NST = cfg.NSH, cfg.NTOT, cfg.WROWS, cfg.NST
    bf16, f32, i16 = mybir.dt.bfloat16, mybir.dt.float32, mybir.dt.int16
    AF = mybir.ActivationFunctionType
    OP = mybir.AluOpType
    icols = sched.totidx // 16

    nc = bacc.Bacc("TRN2", target_bir_lowering=False, debug=False, num_devices=C)

    # I/O
    tl0_in = nc.dram_tensor("tbl0", [NSH, RW], bf16, kind="ExternalInput")
    ssh_in = nc.dram_tensor("ssh0", [P, T * D], f32, kind="ExternalInput")
    idx_in = nc.dram_tensor("idxs", [P, icols], i16, kind="ExternalInput")
    dl_in = nc.dram_tensor("dstloc", [P, sched.totb], f32, kind="ExternalInput")
    dv_in = nc.dram_tensor("dinv", [P, T], f32, kind="ExternalInput")
    w_in = nc.dram_tensor("Wcat", [D, L * D], f32, kind="ExternalInput")
    gb_in = nc.dram_tensor("gbcat", [D, 2 * L], f32, kind="ExternalInput")
    io_in = nc.dram_tensor("iota", [1, P], bf16, kind="ExternalInput")
    id_in = nc.dram_tensor("ident", [P, P], f32, kind="ExternalInput")
    out_d = nc.dram_tensor("out", [NSH, D], f32, kind="ExternalOutput")

    # internal DRAM
    tloc = nc.dram_tensor("tloc", [NSH, RW], bf16)
    tfull = nc.dram_tensor("tfull", [NTOT, RW], bf16, addr_space="Shared")
    if alias_windows:
        wins = [dram_alias(nc, tfull, f"win{w}", [WROWS, RW], bf16,
                           w * WROWS * RW * 2, addr_space="Shared")
                for w in range(NW)]
    else:  # sim only: offset APs (crash real HW, fine in CoreSim)
        wins = [tfull[w * WROWS:(w + 1) * WROWS, :] for w in range(NW)]
    ar_in = nc.dram_tensor("ar_in", [D + 1, D + 1], f32)
    ar_out = nc.dram_tensor("ar_out", [D + 1, D + 1], f32)
    st_d = nc.dram_tensor("st_d", [2, D], f32)

    # persistent SBUF
    zsh = nc.alloc_sbuf_tensor("zsh", [P, T * (D + 1)], f32).ap()
    ssh = nc.alloc_sbuf_tensor("ssh", [P, T * D], f32).ap()
    idx_sb = nc.alloc_sbuf_tensor("idx_sb", [P, icols], i16).ap()
    dl_sb = nc.alloc_sbuf_tensor("dl_sb", [P, sched.totb], f32).ap()
    dv_sb = nc.alloc_sbuf_tensor("dv_sb", [P, T], f32).ap()
    w_sb = nc.alloc_sbuf_tensor("w_sb", [D, L * D], f32).ap()
    gb_sb = nc.alloc_sbuf_tensor("gb_sb", [D, 2 * L], f32).ap()
    io_bc = nc.alloc_sbuf_tensor("io_bc", [P, P], bf16).ap()
    id_sb = nc.alloc_sbuf_tensor("id_sb", [P, P], f32).ap()
    ones_sb = nc.alloc_sbuf_tensor("ones_sb", [P, 1], f32).ap()
    st_row = nc.alloc_sbuf_tensor("st_row", [2, D], f32).ap()
    st_bc = nc.alloc_sbuf_tensor("st_bc", [P, 2 * D], f32).ap()
    stat_sb = nc.alloc_sbuf_tensor("stat_sb", [D + 1, D + 1], f32).ap()

    def rep0(a, n):
        """AP view of a[:, :1]-like col group replicated n times on a new
        inner stride-0 axis of length P: [128, ncols] -> [128, ncols, P]."""
        return bass.AP(a.tensor, a.offset,
                       [list(a.ap[0]), list(a.ap[1]), [0, P]])

    def iota_rep(n):
        """io_bc [128, P] bf16 -> [128, n, P] with the middle axis stride 0."""
        return bass.AP(io_bc.tensor, io_bc.offset,
                       [list(io_bc.ap[0]), [0, n], list(io_bc.ap[1])])

    with tile.TileContext(nc) as tc:
        with ExitStack() as ctx:
            sb = ctx.enter_context(tc.tile_pool(name="sb", bufs=3))
            egp = ctx.enter_context(tc.tile_pool(
                name="eg", bufs=int(os.environ.get("EGB", "6"))))
            mp = ctx.enter_context(tc.tile_pool(name="mp", bufs=4))
            wp = ctx.enter_context(tc.tile_pool(name="wp", bufs=2))
            pp = ctx.enter_context(tc.tile_pool(name="pp", bufs=3, space="PSUM"))
            pt = ctx.enter_context(tc.tile_pool(name="pt", bufs=2, space="PSUM"))
            pz = ctx.enter_context(tc.tile_pool(name="pz", bufs=2, space="PSUM"))
            ps = ctx.enter_context(tc.tile_pool(name="ps", bufs=1, space="PSUM"))

            for _rep in range(reps):
                # ---- prologue: consts + layer-0 table (host-prepared) ----
                nc.sync.dma_start(out=idx_sb[:], in_=idx_in[:, :])
                nc.sync.dma_start(out=dl_sb[:], in_=dl_in[:, :])
                nc.sync.dma_start(out=dv_sb[:], in_=dv_in[:, :])
                nc.sync.dma_start(out=w_sb[:], in_=w_in[:, :])
                nc.sync.dma_start(out=gb_sb[:], in_=gb_in[:, :])
                nc.gpsimd.dma_start(out=io_bc[:], in_=bass.AP(
                    io_in.ap().tensor, 0, [[0, P], [1, P]]))
                nc.sync.dma_start(out=id_sb[:], in_=id_in[:, :])
                nc.vector.memset(ones_sb[:], 1.0)
                nc.vector.memset(
                    zsh.rearrange("p (t e) -> p t e", t=T)[:, :, D:D + 1], 1.0)
                nc.sync.dma_start(out=ssh[:], in_=ssh_in[:, :])
                nc.sync.dma_start(out=tloc[:, :], in_=tl0_in[:, :])

                # ---- layers ----
                for l in range(DBG_LAYERS):
                    nc.gpsimd.collective_compute(
                        "AllGather", OP.bypass,
                        ins=[tloc.ap().opt()],
                        outs=[tfull.ap().opt()],
                        replica_groups=[list(range(C))],
                    )
                    tc.strict_bb_all_engine_barrier()   # alias deps: tfull -> wins
                    gram_ps = ps.tile([D + 1, D + 1], f32, tag="stats")
                    gi = 0  # group index
                    if DBG_SKIP_GATHER:
                        nc.sync.dma_start(out=out_d[0:P, :], in_=st_bc[:, 0:D])
                        continue
                    for st in range(NST):
                        ts = list(range(st * G, min((st + 1) * G, T)))
                        egs = {}
                        for w in range(NW):
                            ng = sched.grp_ng[gi + w]
                            if ng == 0:
                                continue
                            eg = egp.tile([P, ng * RW], bf16, tag="eg")
                            eg3 = eg[:].rearrange("p (b e) -> p b e", b=ng)
                            ioff = sched.grp_ioff[gi + w]
                            # chunk the group's blocks into dma_gather calls
                            for g0 in range(0, ng, NIDX_BLKS):
                                nb2 = min(NIDX_BLKS, ng - g0)
                                nc.gpsimd.dma_gather(
                                    out_ap=eg3[:, g0:g0 + nb2, :],
                                    in_ap=wins[w][:, :] if alias_windows else wins[w],
                                    idxs_ap=idx_sb[:, ioff + g0 * 8:
                                                   ioff + (g0 + nb2) * 8],
                                    num_idxs=nb2 * P,
                                    num_idxs_reg=nb2 * P,
                                    elem_size=RW,
                                )
                            egs[w] = eg
                        if DBG_SKIP_MM:
                            continue

                        # M blocks for this supertile
                        c0 = sched.blk_col[ts[0], 0]
                        c1 = c0 + sum(sched.grp_ng[gi + w] for w in range(NW))
                        mts = {}
                        for mb0 in range(c0, c1, MB):
                            nb = min(MB, c1 - mb0)
                            mt = mp.tile([P, MB * P], bf16, tag="m")
                            if BATCH_M:
                                mt3 = mt[:].rearrange("p (b c) -> p b c", b=MB)
                                a = dl_sb[:, mb0:mb0 + nb]
                                nc.vector.tensor_tensor(
                                    out=mt3[:, :nb, :],
                                    in0=rep0(a, nb),
                                    in1=iota_rep(nb),
                                    op=OP.is_equal)
                            else:
                                for j in range(nb):
                                    nc.vector.tensor_scalar(
                                        out=mt[:, j * P:(j + 1) * P],
                                        in0=io_bc[:],
                                        scalar1=dl_sb[:, mb0 + j:mb0 + j + 1],
                                        scalar2=None, op0=OP.is_equal)
                            for j in range(nb):
                                mts[mb0 + j] = (mt, j)

                        for t in ts:
                            agg = pp.tile([P, RW], f32, tag="agg")
                            for w in range(NW):
                                for b in range(sched.B[t, w]):
                                    col = sched.blk_col[t, w] + b
                                    mt, j = mts[col]
                                    bidx = sched.eg_boff[t, w] + b
                                    nc.tensor.matmul(
                                        out=agg[:],
                                        lhsT=mt[:, j * P:(j + 1) * P],
                                        rhs=egs[w][:, bidx * RW:(bidx + 1) * RW],
                                        start=(w == sched.tb_first[t] and b == 0),
                                        stop=(w == sched.tb_last[t]
                                              and b == sched.B[t, w] - 1),
                                    )
                            # acc2 = (hi + lo) * dinv + self  (dinv fused into
                            # the PSUM->SBUF copy; TT cannot read 2 PSUM ops)
                            agf = sb.tile([P, RW], f32, tag="agf")
                            nc.scalar.activation(agf[:], agg[:], AF.Copy,
                                                 scale=dv_sb[:, t:t + 1])
                            acc2a = sb.tile([P, D], f32, tag="acc2a")
                            nc.vector.tensor_tensor(
                                out=acc2a[:], in0=agf[:, :D], in1=agf[:, D:],
                                op=OP.add)
                            acc2 = sb.tile([P, D], f32, tag="acc2")
                            nc.vector.tensor_tensor(
                                out=acc2[:], in0=acc2a[:],
                                in1=ssh[:, t * D:(t + 1) * D], op=OP.add)
                            # z = acc2 @ W_l  (via PE transpose)
                            trp = pt.tile([D, P], f32, tag="tr")
                            nc.tensor.transpose(trp[:], acc2[:], id_sb[:])
                            accT = sb.tile([D, P], f32, tag="accT")
                            nc.scalar.activation(accT[:], trp[:], AF.Copy)
                            zp = pz.tile([P, D], f32, tag="z")
                            nc.tensor.matmul(
                                out=zp[:], lhsT=accT[:],
                                rhs=w_sb[:, l * D:(l + 1) * D],
                                start=True, stop=True)
                            nc.scalar.activation(
                                zsh[:, t * (D + 1):t * (D + 1) + D], zp[:],
                                AF.Copy)
                            # stats: [z|1].T @ [z|1] -> gram + sums in one chain
                            zaug = zsh[:, t * (D + 1):(t + 1) * (D + 1)]
                            nc.tensor.matmul(
                                out=gram_ps[:],
                                lhsT=zaug, rhs=zaug,
                                start=(t == 0), stop=(t == T - 1))
                        gi += NW

                    # ---- BN stats: AllReduce + affine coefficients ----
                    nc.vector.tensor_copy(out=stat_sb[:], in_=gram_ps[:])
                    if not DBG_SKIP_AR:
                        nc.sync.dma_start(out=ar_in[:, :], in_=stat_sb[:])
                        nc.gpsimd.collective_compute(
                            "AllReduce", OP.add,
                            ins=[ar_in.ap().opt()],
                            outs=[ar_out.ap().opt()],
                            replica_groups=[list(range(C))],
                        )
                        nc.sync.dma_start(out=stat_sb[:], in_=ar_out[:, :])

                    mu = sb.tile([D, 1], f32, tag="v1")
                    nc.vector.tensor_scalar(out=mu[:], in0=stat_sb[:D, D:D + 1],
                                            scalar1=1.0 / cfg.N, scalar2=None,
                                            op0=OP.mult)
                    dg = sb.tile([D, D], f32, tag="dg")
                    nc.vector.tensor_tensor(out=dg[:], in0=stat_sb[:D, :D],
                                            in1=id_sb[:D, :D], op=OP.mult)
                    msq = sb.tile([D, 1], f32, tag="v2")
                    nc.vector.tensor_reduce(out=msq[:], in_=dg[:],
                                            axis=mybir.AxisListType.X, op=OP.add)
                    var = sb.tile([D, 1], f32, tag="v3")
                    # var = msq/N - mu^2 ; then rstd = 1/sqrt(var+eps)
                    musq = sb.tile([D, 1], f32, tag="v4")
                    nc.vector.tensor_tensor(out=musq[:], in0=mu[:], in1=mu[:],
                                            op=OP.mult)
                    nc.vector.tensor_scalar(out=var[:], in0=msq[:],
                                            scalar1=1.0 / cfg.N, scalar2=None,
                                            op0=OP.mult)
                    nc.vector.tensor_tensor(out=var[:], in0=var[:], in1=musq[:],
                                            op=OP.subtract)
                    sd = sb.tile([D, 1], f32, tag="v5")
                    nc.vector.tensor_scalar(out=sd[:], in0=var[:], scalar1=EPS,
                                            scalar2=None, op0=OP.add)
                    nc.scalar.activation(sd[:], sd[:], AF.Sqrt)
                    rstd = sb.tile([D, 1], f32, tag="v6")
                    nc.vector.reciprocal(rstd[:], sd[:])
                    stc = sb.tile([D, 2], f32, tag="stc")
                    nc.vector.tensor_tensor(out=stc[:, 0:1], in0=gb_sb[:, l:l + 1],
                                            in1=rstd[:], op=OP.mult)
                    nc.vector.tensor_tensor(out=stc[:, 1:2], in0=mu[:],
                                            in1=stc[:, 0:1], op=OP.mult)
                    nc.vector.tensor_tensor(out=stc[:, 1:2],
                                            in0=gb_sb[:, L + l:L + l + 1],
                                            in1=stc[:, 1:2], op=OP.subtract)
                    stp = pt.tile([2, D], f32, tag="tr")
                    nc.tensor.transpose(stp[:], stc[:], id_sb[:D, :D])
                    nc.scalar.activation(st_row[:], stp[:], AF.Copy)
                    nc.sync.dma_start(out=st_d[:, :], in_=st_row[:])
                    nc.gpsimd.dma_start(out=st_bc[:], in_=bass.AP(
                        st_d.ap().tensor, 0, [[0, P], [D, 2], [1, D]]))

                    # ---- pass E: affine (+relu, + next table/self strip) ----
                    srow = st_bc[:, 0:D]
                    trow = st_bc[:, D:2 * D]
                    for t0 in range(0, T, BW):
                        nb = min(BW, T - t0)
                        if l < L - 1:
                            wt = wp.tile([P, BW * RW], bf16, tag="wt")
                        else:
                            wt = wp.tile([P, BW * D], f32, tag="wo")
                        for k in range(nb):
                            t = t0 + k
                            zt = zsh[:, t * (D + 1):t * (D + 1) + D]
                            r1 = sb.tile([P, D], f32, tag="r1")
                            nc.vector.tensor_tensor(out=r1[:], in0=zt, in1=srow,
                                                    op=OP.mult)
                            if l < L - 1:
                                r2 = sb.tile([P, D], f32, tag="r2")
                                nc.vector.tensor_tensor(out=r2[:], in0=r1[:],
                                                        in1=trow, op=OP.add)
                                r3 = sb.tile([P, D], f32, tag="r3")
                                nc.vector.tensor_scalar(
                                    out=r3[:], in0=r2[:], scalar1=0.0,
                                    scalar2=dv_sb[:, t:t + 1], op0=OP.max,
                                    op1=OP.mult)
                                # next self strip: dinv * r3
                                nc.vector.tensor_scalar(
                                    out=ssh[:, t * D:(t + 1) * D], in0=r3[:],
                                    scalar1=dv_sb[:, t:t + 1], scalar2=None,
                                    op0=OP.mult)
                                tl = wt[:, k * RW:(k + 1) * RW]
                                nc.scalar.activation(tl[:, :D], r3[:], AF.Copy)
                                nc.vector.tensor_tensor(out=tl[:, D:], in0=r3[:],
                                                        in1=tl[:, :D],
                                                        op=OP.subtract)
                            else:
                                nc.vector.tensor_tensor(
                                    out=wt[:, k * D:(k + 1) * D], in0=r1[:],
                                    in1=trow, op=OP.add)
                        if l < L - 1:
                            nc.sync.dma_start(
                                out=bass.AP(tloc.ap().tensor, t0 * P * RW,
                                            [[RW, P], [P * RW, nb], [1, RW]]),
                                in_=wt[:].rearrange(
                                    "p (b e) -> p b e", b=BW)[:, :nb, :])
                        else:
                            nc.sync.dma_start(
                                out=bass.AP(out_d.ap().tensor, t0 * P * D,
                                            [[D, P], [P * D, nb], [1, D]]),
                                in_=wt[:].rearrange(
                                    "p (b e) -> p b e", b=BW)[:, :nb, :])
    nc.finalize()
    return nc


_PROG_CACHE = {}


def prepare(cfg: Cfg, x, edge_index, Ws, gammas, betas, alias_windows=True,
            reps=1):
    """Host preprocess + program build.  Returns (nc, in_maps, unshard)."""
    sched, xs, idxs, dstlocs, dinvs, sshs, core_of, local_of = preprocess(
        cfg, np.asarray(x, np.float32), np.asarray(edge_index))
    Wcat = np.ascontiguousarray(
        np.concatenate([np.asarray(Ws[i], np.float32) for i in range(L)], axis=1))
    gbcat = np.ascontiguousarray(np.stack(
        [np.asarray(gammas[i], np.float32) for i in range(L)]
        + [np.asarray(betas[i], np.float32) for i in range(L)], axis=1))
    iota = np.arange(P, dtype=np.float32).reshape(1, P).astype(ml_dtypes.bfloat16)
    ident = np.eye(P, dtype=np.float32)

    key = (cfg.N, cfg.T, cfg.G, sched.totb, sched.totidx,
           tuple(sched.grp_ng), sched.B.tobytes(), alias_windows, reps)
    if key not in _PROG_CACHE:
        _PROG_CACHE[key] = build_program(cfg, sched, alias_windows=alias_windows,
                                         reps=reps)
    nc = _PROG_CACHE[key]

    in_maps = []
    for c in range(C):
        in_maps.append({
            "tbl0": xs[c], "ssh0": sshs[c], "idxs": idxs[c],
            "dstloc": dstlocs[c], "dinv": dinvs[c], "Wcat": Wcat,
            "gbcat": gbcat, "iota": iota, "ident": ident,
        })

    def unshard(outs_by_core):
        outs = np.stack([np.asarray(o) for o in outs_by_core])
        return outs[core_of, local_of]          # [N, 64]

    return nc, in_maps, unshard


def run(cfg: Cfg, x, edge_index, Ws, gammas, betas, trace=False):
    nc, in_maps, unshard = prepare(cfg, x, edge_index, Ws, gammas, betas)
    res = run_bass_kernel_spmd(nc, in_maps, core_ids=list(range(C)),
                               trace=trace)
    out_full = unshard([res.results[c]["out"] for c in range(C)])
    return out_full, res


def kernel(x, edge_index, Ws, bs, gammas, betas):
    out, _ = run(FULL, x, edge_index, Ws, gammas, betas, trace=False)
    return out
